# revision 33
# baseline (speedup 1.0000x reference)
"""Causal self-attention (B=4, T=2048, C=1024, H=16) on 8 Trainium2 NeuronCores.

Sharding: core = (b, g) with b = core//2 (batch), g = core%2 (head group of 8
heads / 512 features).  Each core computes its batch's attention for its 8
heads plus the partial output projection for its feature slice; the host sums
the two partials per batch and adds the projection bias.

Per-core kernel (all shapes hardcoded):
  inputs  xT (1024, 2048) = x[b].T          bf16
          wqT/wkT/wvT (1024, 512) = W[g].T  bf16
          wpT (512, 1024) = Wp[:, g].T      bf16
          bqh/bkh (128, 4), bvh (1, 512)    fp32
  output  out (2048, 1024) partial          fp32

Pipeline is zippered per 512-wide t-block: QKV for block nb, then attention
for i-block nb (which only needs K/V up to the diagonal), then the previous
block's projection — so the PE-bound QKV work overlaps the ScalarE-bound
softmax stretches.  Scores for a head pair run as two K=64 matmuls in
different PE row groups (concurrent).  AV multiplies [V_h | ones64].T @ E so
PSUM rows 64:128 hold the softmax denominator, normalized with a fast
approximate reciprocal.  The causal mask costs nothing off the diagonal
(blocks above the diagonal are skipped) and a triangular bf16 mask on it.
"""

import sys

if "/opt/trn_rl_repo" not in sys.path:
    sys.path.insert(0, "/opt/trn_rl_repo")

import numpy as np

B, T, C, H = 4, 2048, 1024, 16
D = C // H          # 64 head dim
GH = H // 2         # 8 heads per core
CG = C // 2         # 512 features per head group
P = 128             # partitions
NBLK = 512          # free-dim block (t-block / i-block)
N_CORES = 8

_CACHE = {}
RUN_KWARGS = {}     # test harness can set {"trace": True, ...}
LAST_RESULT = [None]


def _build_nc(t=T):
    import concourse.mybir as mybir
    from concourse import bacc
    from concourse.tile import TileContext
    from contextlib import ExitStack

    f32 = mybir.dt.float32
    bf16 = mybir.dt.bfloat16
    Exp = mybir.ActivationFunctionType.Exp

    nt = t // P            # t-tiles
    nib = t // NBLK        # i-blocks / t-blocks
    ck = C // P            # 8 contraction tiles over C
    nm = CG // P           # 4 c'-tiles per group
    blk_t = NBLK // P      # 4 t-tiles per block

    nc = bacc.Bacc("TRN2", target_bir_lowering=False, num_devices=N_CORES)

    xT = nc.dram_tensor("xT", (C, t), bf16, kind="ExternalInput")
    wqT = nc.dram_tensor("wqT", (C, CG), bf16, kind="ExternalInput")
    wkT = nc.dram_tensor("wkT", (C, CG), bf16, kind="ExternalInput")
    wvT = nc.dram_tensor("wvT", (C, CG), bf16, kind="ExternalInput")
    wpT = nc.dram_tensor("wpT", (CG, C), bf16, kind="ExternalInput")
    bqh = nc.dram_tensor("bqh", (P, nm), f32, kind="ExternalInput")
    bkh = nc.dram_tensor("bkh", (P, nm), f32, kind="ExternalInput")
    bvh = nc.dram_tensor("bvh", (1, CG), f32, kind="ExternalInput")
    out = nc.dram_tensor("out", (t, C), f32, kind="ExternalOutput")

    with TileContext(nc) as tc, ExitStack() as es:
        pp = es.enter_context(tc.tile_pool(name="persist", bufs=1))
        epool = es.enter_context(tc.tile_pool(name="e", bufs=10))
        ytpool = es.enter_context(tc.tile_pool(name="yt", bufs=8))
        opool = es.enter_context(tc.tile_pool(name="osb", bufs=4))
        npool = es.enter_context(tc.tile_pool(name="nrm", bufs=3))
        qkvpool = es.enter_context(tc.tile_pool(name="qkv_ps", bufs=1, space="PSUM"))
        stpool = es.enter_context(tc.tile_pool(name="st_ps", bufs=2, space="PSUM"))
        avpool = es.enter_context(tc.tile_pool(name="av_ps", bufs=3, space="PSUM"))
        pjpool = avpool

        # ---- all of x, resident (32KB/partition bf16), one DMA per t-block ----
        # layout: columns grouped as (nb, k, NBLK); src xT[(k p), t] rearranged
        x_all = pp.tile([P, nib * ck * NBLK], bf16, tag="x_all", name="x_all")
        xT_r = xT.rearrange("(k p) t -> p k t", p=P)

        def load_x_block(nb):
            nc.sync.dma_start(
                out=x_all[:, nb * ck * NBLK:(nb + 1) * ck * NBLK],
                in_=xT_r[:, :, nb * NBLK:(nb + 1) * NBLK],
            )

        load_x_block(0)
        x_sb = {(nb, k): x_all[:, (nb * ck + k) * NBLK:(nb * ck + k + 1) * NBLK]
                for nb in range(nib) for k in range(ck)}

        # ---- constants ----
        bq_sb = pp.tile([P, nm], f32, tag="bq_sb", name="bq_sb")
        nc.sync.dma_start(out=bq_sb, in_=bqh[:, :])
        bk_sb = pp.tile([P, nm], f32, tag="bk_sb", name="bk_sb")
        nc.sync.dma_start(out=bk_sb, in_=bkh[:, :])
        bv_row = pp.tile([1, CG], f32, tag="bv_row", name="bv_row")
        nc.sync.dma_start(out=bv_row, in_=bvh[:, :])
        bv_bc = pp.tile([P, CG], f32, tag="bv_bc", name="bv_bc")
        nc.gpsimd.partition_broadcast(bv_bc, bv_row)
        # tri[p, y] = 1 if y >= p else 0  (keep i_local >= j_local)
        tri = pp.tile([P, P], bf16, tag="tri", name="tri")
        nc.gpsimd.memset(tri, 1.0)
        nc.gpsimd.affine_select(
            out=tri, in_=tri, compare_op=mybir.AluOpType.is_ge,
            fill=0.0, base=0, pattern=[[1, P]], channel_multiplier=-1,
        )

        # ---- weights: one big strided DMA each (wp later: projection-only) ----
        def load_w(wt, nm_):
            big = pp.tile([P, ck * CG], bf16, tag=f"{nm_}_all", name=f"{nm_}_all")
            nc.sync.dma_start(out=big, in_=wt.rearrange("(k p) c -> p k c", p=P))
            return [big[:, k * CG:(k + 1) * CG] for k in range(ck)]

        wq_sb = load_w(wqT, "wq")
        wk_sb = load_w(wkT, "wk")
        wv_sb = load_w(wvT, "wv")
        for _nb in range(1, nib):
            load_x_block(_nb)

        qt_sb = {}   # (m, nb) -> (128, 512) bf16 tile of Q^T
        kt_sb = {}
        v_sb = []    # per t-tile (128, 8*128) bf16: per head 64 V cols + 64 ones
        wp_sb = []
        yt_hist = {}

        def qkv_pieces(nb):
            """Return 6 closures: [Q mg0, Q mg1, K mg0, K mg1, V ig0, V ig1].
            The first piece also issues the x-block DMAs."""
            xts = [x_sb[(nb, k)] for k in range(ck)]

            def load_x():
                pass

            def qk_piece(which, mg):
                w_sb = wq_sb if which == "q" else wk_sb
                bias = bq_sb if which == "q" else bk_sb
                tgt = qt_sb if which == "q" else kt_sb

                def run():
                    for i in range(2):
                        m = 2 * mg + i
                        ps = qkvpool.tile([P, NBLK], f32, tag="ps", name=f"ps{which}{nb}_{mg}_{i}")
                        for k in range(ck):
                            nc.tensor.matmul(ps, w_sb[k][:, m * P:(m + 1) * P], xts[k],
                                             start=(k == 0), stop=(k == ck - 1))
                        tl = pp.tile([P, NBLK], bf16, tag=f"{which}t{m}_{nb}",
                                     name=f"{which}t{m}_{nb}")
                        nc.vector.tensor_scalar_add(tl, ps, bias[:, m:m + 1])
                        tgt[(m, nb)] = tl
                return run

            def v_piece(ig):
                def run():
                    for i in range(2):
                        ps = qkvpool.tile([P, NBLK], f32, tag="ps", name=f"psv{nb}_{ig}_{i}")
                        for k in range(ck):
                            nc.tensor.matmul(ps,
                                             xts[k][:, (2 * ig + i) * P:(2 * ig + i + 1) * P],
                                             wv_sb[k], start=(k == 0), stop=(k == ck - 1))
                        tt = nb * blk_t + 2 * ig + i
                        vt = pp.tile([P, GH * 2 * D], bf16, tag=f"v{tt}", name=f"v{tt}")
                        v3 = vt.rearrange("p (g d) -> p g d", d=2 * D)
                        nc.vector.tensor_add(
                            v3[:, :, 0:D],
                            ps.rearrange("p (h d) -> p h d", d=D),
                            bv_bc.rearrange("p (h d) -> p h d", d=D),
                        )
                        nc.vector.memset(v3[:, :, D:2 * D], 1.0)
                        while len(v_sb) <= tt:
                            v_sb.append(None)
                        v_sb[tt] = vt
                return run

            return load_x, [qk_piece("q", 0), qk_piece("q", 1),
                            qk_piece("k", 0), qk_piece("k", 1),
                            v_piece(0), v_piece(1)]

        def emit_attention_pair(ib, pr):
            jt_max = blk_t * (ib + 1)
            ytps = [avpool.tile([P, NBLK], f32, tag="acc", name=f"ytps{ib}_{pr}_{hh}")
                    for hh in range(2)]
            e_store = [[], []]

            def _av(hh, jt):
                h = 2 * pr + hh
                e2 = e_store[hh][jt // 2]
                half = jt % 2
                nc.tensor.matmul(ytps[hh], v_sb[jt][:, h * 2 * D:(h + 1) * 2 * D],
                                 e2[:, half * NBLK:(half + 1) * NBLK],
                                 start=(jt == 0), stop=(jt == jt_max - 1))

            n_pair_j = jt_max // 2
            for u in range(n_pair_j):
                st2s = [stpool.tile([P, 2 * NBLK], f32, tag="st", name=f"st{ib}_{pr}_{hh}_{u}")
                        for hh in range(2)]
                for half in range(2):
                    jt = 2 * u + half
                    for hh in range(2):
                        r = hh * D
                        nc.tensor.matmul(
                            st2s[hh][:, half * NBLK:(half + 1) * NBLK],
                            kt_sb[(pr, jt // blk_t)][r:r + D, (jt % blk_t) * P:(jt % blk_t + 1) * P],
                            qt_sb[(pr, ib)][r:r + D, :],
                            start=True, stop=True,
                        )
                for hh in range(2):
                    e2 = epool.tile([P, 2 * NBLK], bf16, tag="e", name=f"e{ib}_{pr}_{hh}_{u}")
                    if (2 * u + 1) * P - ib * NBLK < 0:
                        nc.scalar.activation(e2, st2s[hh], Exp, scale=0.125)
                    else:
                        for half in range(2):
                            jt = 2 * u + half
                            base = half * NBLK
                            o = jt * P - ib * NBLK
                            if o < 0:
                                nc.scalar.activation(e2[:, base:base + NBLK],
                                                     st2s[hh][:, base:base + NBLK],
                                                     Exp, scale=0.125)
                            else:
                                if o > 0:
                                    nc.vector.memset(e2[:, base:base + o], 0.0)
                                nc.scalar.activation(e2[:, base + o:base + NBLK],
                                                     st2s[hh][:, base + o:base + NBLK],
                                                     Exp, scale=0.125)
                                nc.vector.tensor_mul(e2[:, base + o:base + o + P],
                                                     e2[:, base + o:base + o + P], tri)
                    e_store[hh].append(e2)
                if u > 0:
                    for hh in range(2):
                        for half in range(2):
                            _av(hh, 2 * (u - 1) + half)
            u = n_pair_j - 1
            for hh in range(2):
                for half in range(2):
                    _av(hh, 2 * u + half)
            yt_cur = ytpool.tile([P, NBLK], bf16, tag="yt", name=f"yt{ib}_{pr}")
            for hh in range(2):
                zsb = npool.tile([D, NBLK], f32, tag="zsb", name=f"z{ib}_{pr}_{hh}")
                nc.vector.tensor_copy(out=zsb, in_=ytps[hh][D:2 * D, :])
                recip = npool.tile([D, NBLK], f32, tag="recip", name=f"rc{ib}_{pr}_{hh}")
                nc.vector.reciprocal_approx_fast(out=recip, in_=zsb)
                nc.vector.tensor_mul(yt_cur[hh * D:(hh + 1) * D, :], ytps[hh][0:D, :], recip)
            yt_hist.setdefault(ib, []).append(yt_cur)

        def emit_proj(ib, half=None):
            yts = yt_hist[ib]
            rng = range(blk_t) if half is None else range(half * blk_t // 2, (half + 1) * blk_t // 2)
            for i in rng:
                tt = ib * blk_t + i
                for cb in range(C // NBLK):
                    pj = pjpool.tile([P, NBLK], f32, tag="acc", name=f"pj{tt}_{cb}")
                    for p_ in range(nm):
                        nc.tensor.matmul(pj, yts[p_][:, i * P:(i + 1) * P],
                                         wp_sb[p_][:, cb * NBLK:(cb + 1) * NBLK],
                                         start=(p_ == 0), stop=(p_ == nm - 1))
                    ot = opool.tile([P, NBLK], f32, tag="osb", name=f"ot{tt}_{cb}")
                    nc.scalar.copy(out=ot, in_=pj)
                    nc.gpsimd.dma_start(out=out[tt * P:(tt + 1) * P, cb * NBLK:(cb + 1) * NBLK],
                                        in_=ot)

        for pc in qkv_pieces(0)[1]:
            pc()
        wp_all = pp.tile([P, nm * C], bf16, tag="wp_all", name="wp_all")
        nc.sync.dma_start(out=wp_all, in_=wpT.rearrange("(a p) c -> p a c", p=P))
        wp_sb.extend(wp_all[:, p_ * C:(p_ + 1) * C] for p_ in range(nm))
        for blk in range(nib):
            pieces = qkv_pieces(blk + 1)[1] if blk + 1 < nib else []
            sched = {0: pieces[0:2], 1: pieces[2:4], 2: pieces[4:5], 3: pieces[5:6]}
            for pr in range(GH // 2):
                for pc in sched.get(pr, []):
                    pc()
                emit_attention_pair(blk, pr)
                if blk > 0 and pr in (2, 3):
                    emit_proj(blk - 1, half=pr - 2)
        emit_proj(nib - 1)

    nc.compile()
    return nc


def _get_nc(t=T):
    if t not in _CACHE:
        _CACHE[t] = _build_nc(t)
    return _CACHE[t]


def kernel(x, Wq, bq, Wk, bk, Wv, bv, Wp, bp):
    import ml_dtypes
    from concourse import bass_utils

    x = np.asarray(x, dtype=np.float32)
    Wq = np.asarray(Wq, dtype=np.float32)
    Wk = np.asarray(Wk, dtype=np.float32)
    Wv = np.asarray(Wv, dtype=np.float32)
    Wp = np.asarray(Wp, dtype=np.float32)
    bq = np.asarray(bq, dtype=np.float32)
    bk = np.asarray(bk, dtype=np.float32)
    bv = np.asarray(bv, dtype=np.float32)
    bp = np.asarray(bp, dtype=np.float32)

    nc = _get_nc()
    bf = ml_dtypes.bfloat16

    in_maps = []
    for core in range(N_CORES):
        b, g = core // 2, core % 2
        gs = slice(g * CG, (g + 1) * CG)
        in_maps.append({
            "xT": x[b].T.astype(bf),
            "wqT": Wq[gs, :].T.astype(bf),
            "wkT": Wk[gs, :].T.astype(bf),
            "wvT": Wv[gs, :].T.astype(bf),
            "wpT": Wp[:, gs].T.astype(bf),
            "bqh": np.ascontiguousarray(bq[gs].reshape(CG // P, P).T),
            "bkh": np.ascontiguousarray(bk[gs].reshape(CG // P, P).T),
            "bvh": bv[gs].reshape(1, CG),
        })

    res = bass_utils.run_bass_kernel_spmd(nc, in_maps, core_ids=list(range(N_CORES)),
                                          **RUN_KWARGS)
    LAST_RESULT[0] = res
    y = np.empty((B, T, C), dtype=np.float32)
    for b in range(B):
        y[b] = res.results[2 * b]["out"] + res.results[2 * b + 1]["out"] + bp
    return y


# revision 34
# speedup vs baseline: 1.0381x; 1.0381x over previous
"""Causal self-attention (B=4, T=2048, C=1024, H=16) on 8 Trainium2 NeuronCores.

Sharding: core = (b, g) with b = core//2 (batch), g = core%2 (head group of 8
heads / 512 features).  Each core computes its batch's attention for its 8
heads plus the partial output projection for its feature slice; the host sums
the two partials per batch and adds the projection bias.

Per-core kernel (all shapes hardcoded):
  inputs  xT (1024, 2048) = x[b].T          bf16
          wqT/wkT/wvT (1024, 512) = W[g].T  bf16
          wpT (512, 1024) = Wp[:, g].T      bf16
          bqh/bkh (128, 4), bvh (1, 512)    fp32
  output  out (2048, 1024) partial          fp32

Pipeline is zippered per 512-wide t-block: QKV for block nb, then attention
for i-block nb (which only needs K/V up to the diagonal), then the previous
block's projection — so the PE-bound QKV work overlaps the ScalarE-bound
softmax stretches.  Scores for a head pair run as two K=64 matmuls in
different PE row groups (concurrent).  AV multiplies [V_h | ones64].T @ E so
PSUM rows 64:128 hold the softmax denominator, normalized with a fast
approximate reciprocal.  The causal mask costs nothing off the diagonal
(blocks above the diagonal are skipped) and a triangular bf16 mask on it.
"""

import sys

if "/opt/trn_rl_repo" not in sys.path:
    sys.path.insert(0, "/opt/trn_rl_repo")

import numpy as np

B, T, C, H = 4, 2048, 1024, 16
D = C // H          # 64 head dim
GH = H // 2         # 8 heads per core
CG = C // 2         # 512 features per head group
P = 128             # partitions
NBLK = 512          # free-dim block (t-block / i-block)
N_CORES = 8

_CACHE = {}
RUN_KWARGS = {}     # test harness can set {"trace": True, ...}
LAST_RESULT = [None]


def _build_nc(t=T):
    import concourse.mybir as mybir
    from concourse import bacc
    from concourse.tile import TileContext
    from contextlib import ExitStack

    f32 = mybir.dt.float32
    bf16 = mybir.dt.bfloat16
    Exp = mybir.ActivationFunctionType.Exp

    nt = t // P            # t-tiles
    nib = t // NBLK        # i-blocks / t-blocks
    ck = C // P            # 8 contraction tiles over C
    nm = CG // P           # 4 c'-tiles per group
    blk_t = NBLK // P      # 4 t-tiles per block

    nc = bacc.Bacc("TRN2", target_bir_lowering=False, num_devices=N_CORES)

    xT = nc.dram_tensor("xT", (C, t), bf16, kind="ExternalInput")
    wqT = nc.dram_tensor("wqT", (C, CG), bf16, kind="ExternalInput")
    wkT = nc.dram_tensor("wkT", (C, CG), bf16, kind="ExternalInput")
    wvT = nc.dram_tensor("wvT", (C, CG), bf16, kind="ExternalInput")
    wpT = nc.dram_tensor("wpT", (CG, C), bf16, kind="ExternalInput")
    bqh = nc.dram_tensor("bqh", (P, nm), f32, kind="ExternalInput")
    bkh = nc.dram_tensor("bkh", (P, nm), f32, kind="ExternalInput")
    bvh = nc.dram_tensor("bvh", (1, CG), f32, kind="ExternalInput")
    out = nc.dram_tensor("out", (t, C), f32, kind="ExternalOutput")

    with TileContext(nc) as tc, ExitStack() as es:
        pp = es.enter_context(tc.tile_pool(name="persist", bufs=1))
        epool = es.enter_context(tc.tile_pool(name="e", bufs=10))
        ytpool = es.enter_context(tc.tile_pool(name="yt", bufs=8))
        opool = es.enter_context(tc.tile_pool(name="osb", bufs=4))
        npool = es.enter_context(tc.tile_pool(name="nrm", bufs=3))
        qkvpool = es.enter_context(tc.tile_pool(name="qkv_ps", bufs=1, space="PSUM"))
        stpool = es.enter_context(tc.tile_pool(name="st_ps", bufs=2, space="PSUM"))
        avpool = es.enter_context(tc.tile_pool(name="av_ps", bufs=3, space="PSUM"))
        pjpool = avpool

        # ---- all of x, resident (32KB/partition bf16), one DMA per t-block ----
        # layout: columns grouped as (nb, k, NBLK); src xT[(k p), t] rearranged
        x_all = pp.tile([P, nib * ck * NBLK], bf16, tag="x_all", name="x_all")
        xT_r = xT.rearrange("(k p) t -> p k t", p=P)

        def load_x_block(nb):
            nc.sync.dma_start(
                out=x_all[:, nb * ck * NBLK:(nb + 1) * ck * NBLK],
                in_=xT_r[:, :, nb * NBLK:(nb + 1) * NBLK],
            )

        load_x_block(0)
        x_sb = {(nb, k): x_all[:, (nb * ck + k) * NBLK:(nb * ck + k + 1) * NBLK]
                for nb in range(nib) for k in range(ck)}

        # ---- constants ----
        bq_sb = pp.tile([P, nm], f32, tag="bq_sb", name="bq_sb")
        nc.sync.dma_start(out=bq_sb, in_=bqh[:, :])
        bk_sb = pp.tile([P, nm], f32, tag="bk_sb", name="bk_sb")
        nc.sync.dma_start(out=bk_sb, in_=bkh[:, :])
        bv_row = pp.tile([1, CG], f32, tag="bv_row", name="bv_row")
        nc.sync.dma_start(out=bv_row, in_=bvh[:, :])
        bv_bc = pp.tile([P, CG], f32, tag="bv_bc", name="bv_bc")
        nc.gpsimd.partition_broadcast(bv_bc, bv_row)
        # tri[p, y] = 1 if y >= p else 0  (keep i_local >= j_local)
        tri = pp.tile([P, P], bf16, tag="tri", name="tri")
        nc.gpsimd.memset(tri, 1.0)
        nc.gpsimd.affine_select(
            out=tri, in_=tri, compare_op=mybir.AluOpType.is_ge,
            fill=0.0, base=0, pattern=[[1, P]], channel_multiplier=-1,
        )

        # ---- weights: one big strided DMA each (wp later: projection-only) ----
        def load_w(wt, nm_):
            big = pp.tile([P, ck * CG], bf16, tag=f"{nm_}_all", name=f"{nm_}_all")
            nc.sync.dma_start(out=big, in_=wt.rearrange("(k p) c -> p k c", p=P))
            return [big[:, k * CG:(k + 1) * CG] for k in range(ck)]

        wq_sb = load_w(wqT, "wq")
        wk_sb = load_w(wkT, "wk")
        wv_sb = load_w(wvT, "wv")
        for _nb in range(1, nib):
            load_x_block(_nb)

        qt_sb = {}   # (m, nb) -> (128, 512) bf16 tile of Q^T
        kt_sb = {}
        v_sb = []    # per t-tile (128, 8*128) bf16: per head 64 V cols + 64 ones
        wp_sb = []
        yt_hist = {}

        def qkv_pieces(nb):
            """Return 6 closures: [Q mg0, Q mg1, K mg0, K mg1, V ig0, V ig1].
            The first piece also issues the x-block DMAs."""
            xts = [x_sb[(nb, k)] for k in range(ck)]

            def load_x():
                pass

            def qk_piece(which, mg):
                w_sb = wq_sb if which == "q" else wk_sb
                bias = bq_sb if which == "q" else bk_sb
                tgt = qt_sb if which == "q" else kt_sb

                def run():
                    for i in range(2):
                        m = 2 * mg + i
                        ps = qkvpool.tile([P, NBLK], f32, tag="ps", name=f"ps{which}{nb}_{mg}_{i}")
                        for k in range(ck):
                            nc.tensor.matmul(ps, w_sb[k][:, m * P:(m + 1) * P], xts[k],
                                             start=(k == 0), stop=(k == ck - 1))
                        tl = pp.tile([P, NBLK], bf16, tag=f"{which}t{m}_{nb}",
                                     name=f"{which}t{m}_{nb}")
                        nc.vector.tensor_scalar_add(tl, ps, bias[:, m:m + 1])
                        tgt[(m, nb)] = tl
                return run

            def v_piece(ig):
                def run():
                    for i in range(2):
                        ps = qkvpool.tile([P, NBLK], f32, tag="ps", name=f"psv{nb}_{ig}_{i}")
                        for k in range(ck):
                            nc.tensor.matmul(ps,
                                             xts[k][:, (2 * ig + i) * P:(2 * ig + i + 1) * P],
                                             wv_sb[k], start=(k == 0), stop=(k == ck - 1))
                        tt = nb * blk_t + 2 * ig + i
                        vt = pp.tile([P, GH * 2 * D], bf16, tag=f"v{tt}", name=f"v{tt}")
                        v3 = vt.rearrange("p (g d) -> p g d", d=2 * D)
                        nc.vector.tensor_add(
                            v3[:, :, 0:D],
                            ps.rearrange("p (h d) -> p h d", d=D),
                            bv_bc.rearrange("p (h d) -> p h d", d=D),
                        )
                        nc.vector.memset(v3[:, :, D:2 * D], 1.0)
                        while len(v_sb) <= tt:
                            v_sb.append(None)
                        v_sb[tt] = vt
                return run

            return load_x, [qk_piece("q", 0), qk_piece("q", 1),
                            qk_piece("k", 0), qk_piece("k", 1),
                            v_piece(0), v_piece(1)]

        def emit_attention_pair(ib, pr):
            jt_max = blk_t * (ib + 1)
            ytps = [avpool.tile([P, NBLK], f32, tag="acc", name=f"ytps{ib}_{pr}_{hh}")
                    for hh in range(2)]
            e_store = [[], []]

            def _av(hh, jt):
                h = 2 * pr + hh
                e2 = e_store[hh][jt // 2]
                half = jt % 2
                nc.tensor.matmul(ytps[hh], v_sb[jt][:, h * 2 * D:(h + 1) * 2 * D],
                                 e2[:, half * NBLK:(half + 1) * NBLK],
                                 start=(jt == 0), stop=(jt == jt_max - 1))

            n_pair_j = jt_max // 2
            for u in range(n_pair_j):
                st2s = [stpool.tile([P, 2 * NBLK], f32, tag="st", name=f"st{ib}_{pr}_{hh}_{u}")
                        for hh in range(2)]
                for half in range(2):
                    jt = 2 * u + half
                    for hh in range(2):
                        r = hh * D
                        nc.tensor.matmul(
                            st2s[hh][:, half * NBLK:(half + 1) * NBLK],
                            kt_sb[(pr, jt // blk_t)][r:r + D, (jt % blk_t) * P:(jt % blk_t + 1) * P],
                            qt_sb[(pr, ib)][r:r + D, :],
                            start=True, stop=True,
                        )
                for hh in range(2):
                    e2 = epool.tile([P, 2 * NBLK], bf16, tag="e", name=f"e{ib}_{pr}_{hh}_{u}")
                    if (2 * u + 1) * P - ib * NBLK < 0:
                        nc.scalar.activation(e2, st2s[hh], Exp, scale=0.125)
                    else:
                        for half in range(2):
                            jt = 2 * u + half
                            base = half * NBLK
                            o = jt * P - ib * NBLK
                            if o < 0:
                                nc.scalar.activation(e2[:, base:base + NBLK],
                                                     st2s[hh][:, base:base + NBLK],
                                                     Exp, scale=0.125)
                            else:
                                if o > 0:
                                    nc.vector.memset(e2[:, base:base + o], 0.0)
                                nc.scalar.activation(e2[:, base + o:base + NBLK],
                                                     st2s[hh][:, base + o:base + NBLK],
                                                     Exp, scale=0.125)
                                nc.vector.tensor_mul(e2[:, base + o:base + o + P],
                                                     e2[:, base + o:base + o + P], tri)
                    e_store[hh].append(e2)
                if u > 0:
                    for hh in range(2):
                        for half in range(2):
                            _av(hh, 2 * (u - 1) + half)
            u = n_pair_j - 1
            for hh in range(2):
                for half in range(2):
                    _av(hh, 2 * u + half)
            yt_cur = ytpool.tile([P, NBLK], bf16, tag="yt", name=f"yt{ib}_{pr}")
            for hh in range(2):
                zsb = npool.tile([D, NBLK], f32, tag="zsb", name=f"z{ib}_{pr}_{hh}")
                nc.vector.tensor_copy(out=zsb, in_=ytps[hh][D:2 * D, :])
                recip = npool.tile([D, NBLK], f32, tag="recip", name=f"rc{ib}_{pr}_{hh}")
                nc.vector.reciprocal_approx_fast(out=recip, in_=zsb)
                nc.vector.tensor_mul(yt_cur[hh * D:(hh + 1) * D, :], ytps[hh][0:D, :], recip)
            yt_hist.setdefault(ib, []).append(yt_cur)

        def emit_proj(ib):
            yts = yt_hist[ib]
            for i in range(blk_t):
                tt = ib * blk_t + i
                for cb in range(C // NBLK):
                    pj = pjpool.tile([P, NBLK], f32, tag="acc", name=f"pj{tt}_{cb}")
                    for p_ in range(nm):
                        nc.tensor.matmul(pj, yts[p_][:, i * P:(i + 1) * P],
                                         wp_sb[p_][:, cb * NBLK:(cb + 1) * NBLK],
                                         start=(p_ == 0), stop=(p_ == nm - 1))
                    ot = opool.tile([P, NBLK], f32, tag="osb", name=f"ot{tt}_{cb}")
                    nc.scalar.copy(out=ot, in_=pj)
                    nc.gpsimd.dma_start(out=out[tt * P:(tt + 1) * P, cb * NBLK:(cb + 1) * NBLK],
                                        in_=ot)

        for pc in qkv_pieces(0)[1]:
            pc()
        wp_all = pp.tile([P, nm * C], bf16, tag="wp_all", name="wp_all")
        nc.sync.dma_start(out=wp_all, in_=wpT.rearrange("(a p) c -> p a c", p=P))
        wp_sb.extend(wp_all[:, p_ * C:(p_ + 1) * C] for p_ in range(nm))
        for blk in range(nib):
            pieces = qkv_pieces(blk + 1)[1] if blk + 1 < nib else []
            sched = {0: pieces[0:2], 1: pieces[2:4], 2: pieces[4:6], 3: []}
            for pr in range(GH // 2):
                for pc in sched.get(pr, []):
                    pc()
                emit_attention_pair(blk, pr)
            if blk > 0:
                emit_proj(blk - 1)
        emit_proj(nib - 1)

    nc.compile()
    return nc


def _get_nc(t=T):
    if t not in _CACHE:
        _CACHE[t] = _build_nc(t)
    return _CACHE[t]


def kernel(x, Wq, bq, Wk, bk, Wv, bv, Wp, bp):
    import ml_dtypes
    from concourse import bass_utils

    x = np.asarray(x, dtype=np.float32)
    Wq = np.asarray(Wq, dtype=np.float32)
    Wk = np.asarray(Wk, dtype=np.float32)
    Wv = np.asarray(Wv, dtype=np.float32)
    Wp = np.asarray(Wp, dtype=np.float32)
    bq = np.asarray(bq, dtype=np.float32)
    bk = np.asarray(bk, dtype=np.float32)
    bv = np.asarray(bv, dtype=np.float32)
    bp = np.asarray(bp, dtype=np.float32)

    nc = _get_nc()
    bf = ml_dtypes.bfloat16

    in_maps = []
    for core in range(N_CORES):
        b, g = core // 2, core % 2
        gs = slice(g * CG, (g + 1) * CG)
        in_maps.append({
            "xT": x[b].T.astype(bf),
            "wqT": Wq[gs, :].T.astype(bf),
            "wkT": Wk[gs, :].T.astype(bf),
            "wvT": Wv[gs, :].T.astype(bf),
            "wpT": Wp[:, gs].T.astype(bf),
            "bqh": np.ascontiguousarray(bq[gs].reshape(CG // P, P).T),
            "bkh": np.ascontiguousarray(bk[gs].reshape(CG // P, P).T),
            "bvh": bv[gs].reshape(1, CG),
        })

    res = bass_utils.run_bass_kernel_spmd(nc, in_maps, core_ids=list(range(N_CORES)),
                                          **RUN_KWARGS)
    LAST_RESULT[0] = res
    y = np.empty((B, T, C), dtype=np.float32)
    for b in range(B):
        y[b] = res.results[2 * b]["out"] + res.results[2 * b + 1]["out"] + bp
    return y


# revision 35
# speedup vs baseline: 1.0554x; 1.0167x over previous
"""Causal self-attention (B=4, T=2048, C=1024, H=16) on 8 Trainium2 NeuronCores.

Sharding: core = (b, g) with b = core//2 (batch), g = core%2 (head group of 8
heads / 512 features).  Each core computes its batch's attention for its 8
heads plus the partial output projection for its feature slice; the host sums
the two partials per batch and adds the projection bias.

Per-core kernel (all shapes hardcoded):
  inputs  xT (1024, 2048) = x[b].T          bf16
          wqT/wkT/wvT (1024, 512) = W[g].T  bf16
          wpT (512, 1024) = Wp[:, g].T      bf16
          bqh/bkh (128, 4), bvh (1, 512)    fp32
  output  out (2048, 1024) partial          fp32

Pipeline is zippered per 512-wide t-block: QKV for block nb, then attention
for i-block nb (which only needs K/V up to the diagonal), then the previous
block's projection — so the PE-bound QKV work overlaps the ScalarE-bound
softmax stretches.  Scores for a head pair run as two K=64 matmuls in
different PE row groups (concurrent).  AV multiplies [V_h | ones64].T @ E so
PSUM rows 64:128 hold the softmax denominator, normalized with a fast
approximate reciprocal.  The causal mask costs nothing off the diagonal
(blocks above the diagonal are skipped) and a triangular bf16 mask on it.
"""

import sys

if "/opt/trn_rl_repo" not in sys.path:
    sys.path.insert(0, "/opt/trn_rl_repo")

import numpy as np

B, T, C, H = 4, 2048, 1024, 16
D = C // H          # 64 head dim
GH = H // 2         # 8 heads per core
CG = C // 2         # 512 features per head group
P = 128             # partitions
NBLK = 512          # free-dim block (t-block / i-block)
N_CORES = 8

_CACHE = {}
RUN_KWARGS = {}     # test harness can set {"trace": True, ...}
LAST_RESULT = [None]


def _build_nc(t=T):
    import concourse.mybir as mybir
    from concourse import bacc
    from concourse.tile import TileContext
    from contextlib import ExitStack

    f32 = mybir.dt.float32
    bf16 = mybir.dt.bfloat16
    Exp = mybir.ActivationFunctionType.Exp

    nt = t // P            # t-tiles
    nib = t // NBLK        # i-blocks / t-blocks
    ck = C // P            # 8 contraction tiles over C
    nm = CG // P           # 4 c'-tiles per group
    blk_t = NBLK // P      # 4 t-tiles per block

    nc = bacc.Bacc("TRN2", target_bir_lowering=False, num_devices=N_CORES)

    xT = nc.dram_tensor("xT", (C, t), bf16, kind="ExternalInput")
    wqT = nc.dram_tensor("wqT", (C, CG), bf16, kind="ExternalInput")
    wkT = nc.dram_tensor("wkT", (C, CG), bf16, kind="ExternalInput")
    wvT = nc.dram_tensor("wvT", (C, CG), bf16, kind="ExternalInput")
    wpT = nc.dram_tensor("wpT", (CG, C), bf16, kind="ExternalInput")
    bqh = nc.dram_tensor("bqh", (P, nm), f32, kind="ExternalInput")
    bkh = nc.dram_tensor("bkh", (P, nm), f32, kind="ExternalInput")
    bvh = nc.dram_tensor("bvh", (1, CG), f32, kind="ExternalInput")
    out = nc.dram_tensor("out", (t, C), f32, kind="ExternalOutput")

    with TileContext(nc) as tc, ExitStack() as es:
        pp = es.enter_context(tc.tile_pool(name="persist", bufs=1))
        epool = es.enter_context(tc.tile_pool(name="e", bufs=10))
        ytpool = es.enter_context(tc.tile_pool(name="yt", bufs=8))
        opool = es.enter_context(tc.tile_pool(name="osb", bufs=4))
        npool = es.enter_context(tc.tile_pool(name="nrm", bufs=3))
        qkvpool = es.enter_context(tc.tile_pool(name="qkv_ps", bufs=1, space="PSUM"))
        stpool = es.enter_context(tc.tile_pool(name="st_ps", bufs=2, space="PSUM"))
        avpool = es.enter_context(tc.tile_pool(name="av_ps", bufs=3, space="PSUM"))
        pjpool = avpool

        # ---- all of x, resident (32KB/partition bf16), one DMA per t-block ----
        # layout: columns grouped as (nb, k, NBLK); src xT[(k p), t] rearranged
        x_all = pp.tile([P, nib * ck * NBLK], bf16, tag="x_all", name="x_all")
        xT_r = xT.rearrange("(k p) t -> p k t", p=P)

        def load_x_block(nb):
            nc.sync.dma_start(
                out=x_all[:, nb * ck * NBLK:(nb + 1) * ck * NBLK],
                in_=xT_r[:, :, nb * NBLK:(nb + 1) * NBLK],
            )

        load_x_block(0)
        x_sb = {(nb, k): x_all[:, (nb * ck + k) * NBLK:(nb * ck + k + 1) * NBLK]
                for nb in range(nib) for k in range(ck)}

        # ---- constants ----
        bq_sb = pp.tile([P, nm], f32, tag="bq_sb", name="bq_sb")
        nc.sync.dma_start(out=bq_sb, in_=bqh[:, :])
        bk_sb = pp.tile([P, nm], f32, tag="bk_sb", name="bk_sb")
        nc.sync.dma_start(out=bk_sb, in_=bkh[:, :])
        bv_row = pp.tile([1, CG], f32, tag="bv_row", name="bv_row")
        nc.sync.dma_start(out=bv_row, in_=bvh[:, :])
        bv_bc = pp.tile([P, CG], f32, tag="bv_bc", name="bv_bc")
        nc.gpsimd.partition_broadcast(bv_bc, bv_row)
        # tri[p, y] = 1 if y >= p else 0  (keep i_local >= j_local)
        tri = pp.tile([P, P], bf16, tag="tri", name="tri")
        nc.gpsimd.memset(tri, 1.0)
        nc.gpsimd.affine_select(
            out=tri, in_=tri, compare_op=mybir.AluOpType.is_ge,
            fill=0.0, base=0, pattern=[[1, P]], channel_multiplier=-1,
        )

        # ---- weights: one big strided DMA each (wp later: projection-only) ----
        def load_w(wt, nm_):
            big = pp.tile([P, ck * CG], bf16, tag=f"{nm_}_all", name=f"{nm_}_all")
            nc.sync.dma_start(out=big, in_=wt.rearrange("(k p) c -> p k c", p=P))
            return [big[:, k * CG:(k + 1) * CG] for k in range(ck)]

        wq_sb = load_w(wqT, "wq")
        wk_sb = load_w(wkT, "wk")
        wv_sb = load_w(wvT, "wv")
        for _nb in range(1, nib):
            load_x_block(_nb)

        qt_sb = {}   # (m, nb) -> (128, 512) bf16 tile of Q^T
        kt_sb = {}
        v_sb = []    # per t-tile (128, 8*128) bf16: per head 64 V cols + 64 ones
        wp_sb = []
        yt_hist = {}

        def qkv_pieces(nb):
            """Return 6 closures: [Q mg0, Q mg1, K mg0, K mg1, V ig0, V ig1].
            The first piece also issues the x-block DMAs."""
            xts = [x_sb[(nb, k)] for k in range(ck)]

            def load_x():
                pass

            def qk_piece(which, mg):
                w_sb = wq_sb if which == "q" else wk_sb
                bias = bq_sb if which == "q" else bk_sb
                tgt = qt_sb if which == "q" else kt_sb

                def run():
                    for i in range(2):
                        m = 2 * mg + i
                        ps = qkvpool.tile([P, NBLK], f32, tag="ps", name=f"ps{which}{nb}_{mg}_{i}")
                        for k in range(ck):
                            nc.tensor.matmul(ps, w_sb[k][:, m * P:(m + 1) * P], xts[k],
                                             start=(k == 0), stop=(k == ck - 1))
                        tl = pp.tile([P, NBLK], bf16, tag=f"{which}t{m}_{nb}",
                                     name=f"{which}t{m}_{nb}")
                        nc.vector.tensor_scalar_add(tl, ps, bias[:, m:m + 1])
                        tgt[(m, nb)] = tl
                return run

            def v_piece(ig):
                def run():
                    for i in range(2):
                        ps = qkvpool.tile([P, NBLK], f32, tag="ps", name=f"psv{nb}_{ig}_{i}")
                        for k in range(ck):
                            nc.tensor.matmul(ps,
                                             xts[k][:, (2 * ig + i) * P:(2 * ig + i + 1) * P],
                                             wv_sb[k], start=(k == 0), stop=(k == ck - 1))
                        tt = nb * blk_t + 2 * ig + i
                        vt = pp.tile([P, GH * 2 * D], bf16, tag=f"v{tt}", name=f"v{tt}")
                        v3 = vt.rearrange("p (g d) -> p g d", d=2 * D)
                        nc.vector.tensor_add(
                            v3[:, :, 0:D],
                            ps.rearrange("p (h d) -> p h d", d=D),
                            bv_bc.rearrange("p (h d) -> p h d", d=D),
                        )
                        nc.vector.memset(v3[:, :, D:2 * D], 1.0)
                        while len(v_sb) <= tt:
                            v_sb.append(None)
                        v_sb[tt] = vt
                return run

            return load_x, [qk_piece("q", 0), qk_piece("q", 1),
                            qk_piece("k", 0), qk_piece("k", 1),
                            v_piece(0), v_piece(1)]

        def emit_attention_pair(ib, pr):
            jt_max = blk_t * (ib + 1)
            ytps = [avpool.tile([P, NBLK], f32, tag="acc", name=f"ytps{ib}_{pr}_{hh}")
                    for hh in range(2)]
            e_store = [[], []]

            def _av(hh, jt):
                h = 2 * pr + hh
                e2 = e_store[hh][jt // 2]
                half = jt % 2
                o = max(jt * P - ib * NBLK, 0)
                nc.tensor.matmul(ytps[hh][:, o:NBLK],
                                 v_sb[jt][:, h * 2 * D:(h + 1) * 2 * D],
                                 e2[:, half * NBLK + o:(half + 1) * NBLK],
                                 start=(jt == 0), stop=(jt == jt_max - 1))

            n_pair_j = jt_max // 2
            for u in range(n_pair_j):
                st2s = [stpool.tile([P, 2 * NBLK], f32, tag="st", name=f"st{ib}_{pr}_{hh}_{u}")
                        for hh in range(2)]
                for half in range(2):
                    jt = 2 * u + half
                    for hh in range(2):
                        r = hh * D
                        nc.tensor.matmul(
                            st2s[hh][:, half * NBLK:(half + 1) * NBLK],
                            kt_sb[(pr, jt // blk_t)][r:r + D, (jt % blk_t) * P:(jt % blk_t + 1) * P],
                            qt_sb[(pr, ib)][r:r + D, :],
                            start=True, stop=True,
                        )
                for hh in range(2):
                    e2 = epool.tile([P, 2 * NBLK], bf16, tag="e", name=f"e{ib}_{pr}_{hh}_{u}")
                    if (2 * u + 1) * P - ib * NBLK < 0:
                        nc.scalar.activation(e2, st2s[hh], Exp, scale=0.125)
                    else:
                        for half in range(2):
                            jt = 2 * u + half
                            base = half * NBLK
                            o = jt * P - ib * NBLK
                            if o < 0:
                                nc.scalar.activation(e2[:, base:base + NBLK],
                                                     st2s[hh][:, base:base + NBLK],
                                                     Exp, scale=0.125)
                            else:
                                nc.scalar.activation(e2[:, base + o:base + NBLK],
                                                     st2s[hh][:, base + o:base + NBLK],
                                                     Exp, scale=0.125)
                                nc.vector.tensor_mul(e2[:, base + o:base + o + P],
                                                     e2[:, base + o:base + o + P], tri)
                    e_store[hh].append(e2)
                if u > 0:
                    for hh in range(2):
                        for half in range(2):
                            _av(hh, 2 * (u - 1) + half)
            u = n_pair_j - 1
            for hh in range(2):
                for half in range(2):
                    _av(hh, 2 * u + half)
            yt_cur = ytpool.tile([P, NBLK], bf16, tag="yt", name=f"yt{ib}_{pr}")
            for hh in range(2):
                zsb = npool.tile([D, NBLK], f32, tag="zsb", name=f"z{ib}_{pr}_{hh}")
                nc.vector.tensor_copy(out=zsb, in_=ytps[hh][D:2 * D, :])
                recip = npool.tile([D, NBLK], f32, tag="recip", name=f"rc{ib}_{pr}_{hh}")
                nc.vector.reciprocal_approx_fast(out=recip, in_=zsb)
                nc.vector.tensor_mul(yt_cur[hh * D:(hh + 1) * D, :], ytps[hh][0:D, :], recip)
            yt_hist.setdefault(ib, []).append(yt_cur)

        def emit_proj(ib):
            yts = yt_hist[ib]
            for i in range(blk_t):
                tt = ib * blk_t + i
                for cb in range(C // NBLK):
                    pj = pjpool.tile([P, NBLK], f32, tag="acc", name=f"pj{tt}_{cb}")
                    for p_ in range(nm):
                        nc.tensor.matmul(pj, yts[p_][:, i * P:(i + 1) * P],
                                         wp_sb[p_][:, cb * NBLK:(cb + 1) * NBLK],
                                         start=(p_ == 0), stop=(p_ == nm - 1))
                    ot = opool.tile([P, NBLK], f32, tag="osb", name=f"ot{tt}_{cb}")
                    nc.scalar.copy(out=ot, in_=pj)
                    nc.gpsimd.dma_start(out=out[tt * P:(tt + 1) * P, cb * NBLK:(cb + 1) * NBLK],
                                        in_=ot)

        for pc in qkv_pieces(0)[1]:
            pc()
        wp_all = pp.tile([P, nm * C], bf16, tag="wp_all", name="wp_all")
        nc.sync.dma_start(out=wp_all, in_=wpT.rearrange("(a p) c -> p a c", p=P))
        wp_sb.extend(wp_all[:, p_ * C:(p_ + 1) * C] for p_ in range(nm))
        for blk in range(nib):
            pieces = qkv_pieces(blk + 1)[1] if blk + 1 < nib else []
            sched = {0: pieces[0:2], 1: pieces[2:4], 2: pieces[4:6], 3: []}
            for pr in range(GH // 2):
                for pc in sched.get(pr, []):
                    pc()
                emit_attention_pair(blk, pr)
            if blk > 0:
                emit_proj(blk - 1)
        emit_proj(nib - 1)

    nc.compile()
    return nc


def _get_nc(t=T):
    if t not in _CACHE:
        _CACHE[t] = _build_nc(t)
    return _CACHE[t]


def kernel(x, Wq, bq, Wk, bk, Wv, bv, Wp, bp):
    import ml_dtypes
    from concourse import bass_utils

    x = np.asarray(x, dtype=np.float32)
    Wq = np.asarray(Wq, dtype=np.float32)
    Wk = np.asarray(Wk, dtype=np.float32)
    Wv = np.asarray(Wv, dtype=np.float32)
    Wp = np.asarray(Wp, dtype=np.float32)
    bq = np.asarray(bq, dtype=np.float32)
    bk = np.asarray(bk, dtype=np.float32)
    bv = np.asarray(bv, dtype=np.float32)
    bp = np.asarray(bp, dtype=np.float32)

    nc = _get_nc()
    bf = ml_dtypes.bfloat16

    in_maps = []
    for core in range(N_CORES):
        b, g = core // 2, core % 2
        gs = slice(g * CG, (g + 1) * CG)
        in_maps.append({
            "xT": x[b].T.astype(bf),
            "wqT": Wq[gs, :].T.astype(bf),
            "wkT": Wk[gs, :].T.astype(bf),
            "wvT": Wv[gs, :].T.astype(bf),
            "wpT": Wp[:, gs].T.astype(bf),
            "bqh": np.ascontiguousarray(bq[gs].reshape(CG // P, P).T),
            "bkh": np.ascontiguousarray(bk[gs].reshape(CG // P, P).T),
            "bvh": bv[gs].reshape(1, CG),
        })

    res = bass_utils.run_bass_kernel_spmd(nc, in_maps, core_ids=list(range(N_CORES)),
                                          **RUN_KWARGS)
    LAST_RESULT[0] = res
    y = np.empty((B, T, C), dtype=np.float32)
    for b in range(B):
        y[b] = res.results[2 * b]["out"] + res.results[2 * b + 1]["out"] + bp
    return y


# revision 36
# speedup vs baseline: 1.0662x; 1.0103x over previous
"""Causal self-attention (B=4, T=2048, C=1024, H=16) on 8 Trainium2 NeuronCores.

Sharding: core = (b, g) with b = core//2 (batch), g = core%2 (head group of 8
heads / 512 features).  Each core computes its batch's attention for its 8
heads plus the partial output projection for its feature slice; the host sums
the two partials per batch and adds the projection bias.

Per-core kernel (all shapes hardcoded):
  inputs  xT (1024, 2048) = x[b].T          bf16
          wqT/wkT/wvT (1024, 512) = W[g].T  bf16
          wpT (512, 1024) = Wp[:, g].T      bf16
          bqh/bkh (128, 4), bvh (1, 512)    fp32
  output  out (2048, 1024) partial          fp32

Pipeline is zippered per 512-wide t-block: QKV for block nb, then attention
for i-block nb (which only needs K/V up to the diagonal), then the previous
block's projection — so the PE-bound QKV work overlaps the ScalarE-bound
softmax stretches.  Scores for a head pair run as two K=64 matmuls in
different PE row groups (concurrent).  AV multiplies [V_h | ones64].T @ E so
PSUM rows 64:128 hold the softmax denominator, normalized with a fast
approximate reciprocal.  The causal mask costs nothing off the diagonal
(blocks above the diagonal are skipped) and a triangular bf16 mask on it.
"""

import sys

if "/opt/trn_rl_repo" not in sys.path:
    sys.path.insert(0, "/opt/trn_rl_repo")

import numpy as np

B, T, C, H = 4, 2048, 1024, 16
D = C // H          # 64 head dim
GH = H // 2         # 8 heads per core
CG = C // 2         # 512 features per head group
P = 128             # partitions
NBLK = 512          # free-dim block (t-block / i-block)
N_CORES = 8

_CACHE = {}
RUN_KWARGS = {}     # test harness can set {"trace": True, ...}
LAST_RESULT = [None]


def _build_nc(t=T):
    import concourse.mybir as mybir
    from concourse import bacc
    from concourse.tile import TileContext
    from contextlib import ExitStack

    f32 = mybir.dt.float32
    bf16 = mybir.dt.bfloat16
    Exp = mybir.ActivationFunctionType.Exp

    nt = t // P            # t-tiles
    nib = t // NBLK        # i-blocks / t-blocks
    ck = C // P            # 8 contraction tiles over C
    nm = CG // P           # 4 c'-tiles per group
    blk_t = NBLK // P      # 4 t-tiles per block

    nc = bacc.Bacc("TRN2", target_bir_lowering=False, num_devices=N_CORES)

    xT = nc.dram_tensor("xT", (C, t), bf16, kind="ExternalInput")
    wqT = nc.dram_tensor("wqT", (C, CG), bf16, kind="ExternalInput")
    wkT = nc.dram_tensor("wkT", (C, CG), bf16, kind="ExternalInput")
    wvT = nc.dram_tensor("wvT", (C, CG), bf16, kind="ExternalInput")
    wpT = nc.dram_tensor("wpT", (CG, C), bf16, kind="ExternalInput")
    bqh = nc.dram_tensor("bqh", (P, nm), f32, kind="ExternalInput")
    bkh = nc.dram_tensor("bkh", (P, nm), f32, kind="ExternalInput")
    bvh = nc.dram_tensor("bvh", (1, CG), f32, kind="ExternalInput")
    out = nc.dram_tensor("out", (t, C), f32, kind="ExternalOutput")

    with TileContext(nc) as tc, ExitStack() as es:
        pp = es.enter_context(tc.tile_pool(name="persist", bufs=1))
        epool = es.enter_context(tc.tile_pool(name="e", bufs=10))
        ytpool = es.enter_context(tc.tile_pool(name="yt", bufs=8))
        opool = es.enter_context(tc.tile_pool(name="osb", bufs=4))
        npool = es.enter_context(tc.tile_pool(name="nrm", bufs=3))
        qkvpool = es.enter_context(tc.tile_pool(name="qkv_ps", bufs=1, space="PSUM"))
        stpool = es.enter_context(tc.tile_pool(name="st_ps", bufs=2, space="PSUM"))
        avpool = es.enter_context(tc.tile_pool(name="av_ps", bufs=3, space="PSUM"))
        pjpool = avpool

        # ---- all of x, resident (32KB/partition bf16), one DMA per t-block ----
        # layout: columns grouped as (nb, k, NBLK); src xT[(k p), t] rearranged
        x_all = pp.tile([P, nib * ck * NBLK], bf16, tag="x_all", name="x_all")
        xT_r = xT.rearrange("(k p) t -> p k t", p=P)

        def load_x_block(nb):
            nc.sync.dma_start(
                out=x_all[:, nb * ck * NBLK:(nb + 1) * ck * NBLK],
                in_=xT_r[:, :, nb * NBLK:(nb + 1) * NBLK],
            )

        load_x_block(0)
        x_sb = {(nb, k): x_all[:, (nb * ck + k) * NBLK:(nb * ck + k + 1) * NBLK]
                for nb in range(nib) for k in range(ck)}

        # ---- constants ----
        bq_sb = pp.tile([P, nm], f32, tag="bq_sb", name="bq_sb")
        nc.sync.dma_start(out=bq_sb, in_=bqh[:, :])
        bk_sb = pp.tile([P, nm], f32, tag="bk_sb", name="bk_sb")
        nc.sync.dma_start(out=bk_sb, in_=bkh[:, :])
        bv_row = pp.tile([1, CG], f32, tag="bv_row", name="bv_row")
        nc.sync.dma_start(out=bv_row, in_=bvh[:, :])
        bv_bc = pp.tile([P, CG], f32, tag="bv_bc", name="bv_bc")
        nc.gpsimd.partition_broadcast(bv_bc, bv_row)
        # tri[p, y] = 1 if y >= p else 0  (keep i_local >= j_local)
        tri = pp.tile([P, P], bf16, tag="tri", name="tri")
        nc.gpsimd.memset(tri, 1.0)
        nc.gpsimd.affine_select(
            out=tri, in_=tri, compare_op=mybir.AluOpType.is_ge,
            fill=0.0, base=0, pattern=[[1, P]], channel_multiplier=-1,
        )

        # ---- weights: one big strided DMA each (wp later: projection-only) ----
        def load_w(wt, nm_):
            big = pp.tile([P, ck * CG], bf16, tag=f"{nm_}_all", name=f"{nm_}_all")
            nc.sync.dma_start(out=big, in_=wt.rearrange("(k p) c -> p k c", p=P))
            return [big[:, k * CG:(k + 1) * CG] for k in range(ck)]

        wq_sb = load_w(wqT, "wq")
        wk_sb = load_w(wkT, "wk")
        wv_sb = load_w(wvT, "wv")
        for _nb in range(1, nib):
            load_x_block(_nb)

        qt_sb = {}   # (m, nb) -> (128, 512) bf16 tile of Q^T
        kt_sb = {}
        v_sb = []    # per t-tile (128, 8*128) bf16: per head 64 V cols + 64 ones
        wp_sb = []
        yt_hist = {}

        def qkv_pieces(nb):
            """Return 6 closures: [Q mg0, Q mg1, K mg0, K mg1, V ig0, V ig1].
            The first piece also issues the x-block DMAs."""
            xts = [x_sb[(nb, k)] for k in range(ck)]

            def load_x():
                pass

            def qk_piece(which, mg):
                w_sb = wq_sb if which == "q" else wk_sb
                bias = bq_sb if which == "q" else bk_sb
                tgt = qt_sb if which == "q" else kt_sb

                def run():
                    for i in range(2):
                        m = 2 * mg + i
                        ps = qkvpool.tile([P, NBLK], f32, tag="ps", name=f"ps{which}{nb}_{mg}_{i}")
                        for k in range(ck):
                            nc.tensor.matmul(ps, w_sb[k][:, m * P:(m + 1) * P], xts[k],
                                             start=(k == 0), stop=(k == ck - 1))
                        tl = pp.tile([P, NBLK], bf16, tag=f"{which}t{m}_{nb}",
                                     name=f"{which}t{m}_{nb}")
                        nc.vector.tensor_scalar_add(tl, ps, bias[:, m:m + 1])
                        tgt[(m, nb)] = tl
                return run

            def v_piece(ig):
                def run():
                    for i in range(2):
                        ps = qkvpool.tile([P, NBLK], f32, tag="ps", name=f"psv{nb}_{ig}_{i}")
                        for k in range(ck):
                            nc.tensor.matmul(ps,
                                             xts[k][:, (2 * ig + i) * P:(2 * ig + i + 1) * P],
                                             wv_sb[k], start=(k == 0), stop=(k == ck - 1))
                        tt = nb * blk_t + 2 * ig + i
                        vt = pp.tile([P, GH * 2 * D], bf16, tag=f"v{tt}", name=f"v{tt}")
                        v3 = vt.rearrange("p (g d) -> p g d", d=2 * D)
                        nc.vector.tensor_add(
                            v3[:, :, 0:D],
                            ps.rearrange("p (h d) -> p h d", d=D),
                            bv_bc.rearrange("p (h d) -> p h d", d=D),
                        )
                        nc.vector.memset(v3[:, :, D:2 * D], 1.0)
                        while len(v_sb) <= tt:
                            v_sb.append(None)
                        v_sb[tt] = vt
                return run

            return load_x, [qk_piece("q", 0), qk_piece("q", 1),
                            qk_piece("k", 0), qk_piece("k", 1),
                            v_piece(0), v_piece(1)]

        def emit_attention_pair(ib, pr):
            jt_max = blk_t * (ib + 1)
            ytps = [avpool.tile([P, NBLK], f32, tag="acc", name=f"ytps{ib}_{pr}_{hh}")
                    for hh in range(2)]
            e_store = [[], []]

            def _av(hh, jt):
                h = 2 * pr + hh
                e2 = e_store[hh][jt // 2]
                half = jt % 2
                o = max(jt * P - ib * NBLK, 0)
                nc.tensor.matmul(ytps[hh][:, o:NBLK],
                                 v_sb[jt][:, h * 2 * D:(h + 1) * 2 * D],
                                 e2[:, half * NBLK + o:(half + 1) * NBLK],
                                 start=(jt == 0), stop=(jt == jt_max - 1))

            n_pair_j = jt_max // 2
            for u in range(n_pair_j):
                st2s = [stpool.tile([P, 2 * NBLK], f32, tag="st", name=f"st{ib}_{pr}_{hh}_{u}")
                        for hh in range(2)]
                for half in range(2):
                    jt = 2 * u + half
                    for hh in range(2):
                        r = hh * D
                        nc.tensor.matmul(
                            st2s[hh][:, half * NBLK:(half + 1) * NBLK],
                            kt_sb[(pr, jt // blk_t)][r:r + D, (jt % blk_t) * P:(jt % blk_t + 1) * P],
                            qt_sb[(pr, ib)][r:r + D, :],
                            start=True, stop=True,
                        )
                for hh in range(2):
                    e2 = epool.tile([P, 2 * NBLK], bf16, tag="e", name=f"e{ib}_{pr}_{hh}_{u}")
                    if (2 * u + 1) * P - ib * NBLK < 0:
                        nc.scalar.activation(e2, st2s[hh], Exp, scale=0.125)
                    else:
                        for half in range(2):
                            jt = 2 * u + half
                            base = half * NBLK
                            o = jt * P - ib * NBLK
                            if o < 0:
                                nc.scalar.activation(e2[:, base:base + NBLK],
                                                     st2s[hh][:, base:base + NBLK],
                                                     Exp, scale=0.125)
                            else:
                                nc.scalar.activation(e2[:, base + o:base + NBLK],
                                                     st2s[hh][:, base + o:base + NBLK],
                                                     Exp, scale=0.125)
                                nc.vector.tensor_mul(e2[:, base + o:base + o + P],
                                                     e2[:, base + o:base + o + P], tri)
                    e_store[hh].append(e2)
                if u > 0:
                    for hh in range(2):
                        for half in range(2):
                            _av(hh, 2 * (u - 1) + half)
            u = n_pair_j - 1
            for hh in range(2):
                for half in range(2):
                    _av(hh, 2 * u + half)
            yt_cur = ytpool.tile([P, NBLK], bf16, tag="yt", name=f"yt{ib}_{pr}")
            for hh in range(2):
                zsb = npool.tile([D, NBLK], f32, tag="zsb", name=f"z{ib}_{pr}_{hh}")
                nc.vector.tensor_copy(out=zsb, in_=ytps[hh][D:2 * D, :])
                recip = npool.tile([D, NBLK], f32, tag="recip", name=f"rc{ib}_{pr}_{hh}")
                nc.vector.reciprocal_approx_fast(out=recip, in_=zsb)
                nc.vector.tensor_mul(yt_cur[hh * D:(hh + 1) * D, :], ytps[hh][0:D, :], recip)
            yt_hist.setdefault(ib, []).append(yt_cur)

        def emit_proj(ib):
            yts = yt_hist[ib]
            for i in range(blk_t):
                tt = ib * blk_t + i
                for cb in range(C // NBLK):
                    pj = pjpool.tile([P, NBLK], f32, tag="acc", name=f"pj{tt}_{cb}")
                    for p_ in range(nm):
                        nc.tensor.matmul(pj, yts[p_][:, i * P:(i + 1) * P],
                                         wp_sb[p_][:, cb * NBLK:(cb + 1) * NBLK],
                                         start=(p_ == 0), stop=(p_ == nm - 1))
                    ot = opool.tile([P, NBLK], f32, tag="osb", name=f"ot{tt}_{cb}")
                    nc.vector.tensor_copy(out=ot, in_=pj)
                    nc.gpsimd.dma_start(out=out[tt * P:(tt + 1) * P, cb * NBLK:(cb + 1) * NBLK],
                                        in_=ot)

        for pc in qkv_pieces(0)[1]:
            pc()
        wp_all = pp.tile([P, nm * C], bf16, tag="wp_all", name="wp_all")
        nc.sync.dma_start(out=wp_all, in_=wpT.rearrange("(a p) c -> p a c", p=P))
        wp_sb.extend(wp_all[:, p_ * C:(p_ + 1) * C] for p_ in range(nm))
        for blk in range(nib):
            pieces = qkv_pieces(blk + 1)[1] if blk + 1 < nib else []
            sched = {0: pieces[0:2], 1: pieces[2:4], 2: pieces[4:6], 3: []}
            for pr in range(GH // 2):
                for pc in sched.get(pr, []):
                    pc()
                emit_attention_pair(blk, pr)
            if blk > 0:
                emit_proj(blk - 1)
        emit_proj(nib - 1)

    nc.compile()
    return nc


def _get_nc(t=T):
    if t not in _CACHE:
        _CACHE[t] = _build_nc(t)
    return _CACHE[t]


def kernel(x, Wq, bq, Wk, bk, Wv, bv, Wp, bp):
    import ml_dtypes
    from concourse import bass_utils

    x = np.asarray(x, dtype=np.float32)
    Wq = np.asarray(Wq, dtype=np.float32)
    Wk = np.asarray(Wk, dtype=np.float32)
    Wv = np.asarray(Wv, dtype=np.float32)
    Wp = np.asarray(Wp, dtype=np.float32)
    bq = np.asarray(bq, dtype=np.float32)
    bk = np.asarray(bk, dtype=np.float32)
    bv = np.asarray(bv, dtype=np.float32)
    bp = np.asarray(bp, dtype=np.float32)

    nc = _get_nc()
    bf = ml_dtypes.bfloat16

    in_maps = []
    for core in range(N_CORES):
        b, g = core // 2, core % 2
        gs = slice(g * CG, (g + 1) * CG)
        in_maps.append({
            "xT": x[b].T.astype(bf),
            "wqT": Wq[gs, :].T.astype(bf),
            "wkT": Wk[gs, :].T.astype(bf),
            "wvT": Wv[gs, :].T.astype(bf),
            "wpT": Wp[:, gs].T.astype(bf),
            "bqh": np.ascontiguousarray(bq[gs].reshape(CG // P, P).T),
            "bkh": np.ascontiguousarray(bk[gs].reshape(CG // P, P).T),
            "bvh": bv[gs].reshape(1, CG),
        })

    res = bass_utils.run_bass_kernel_spmd(nc, in_maps, core_ids=list(range(N_CORES)),
                                          **RUN_KWARGS)
    LAST_RESULT[0] = res
    y = np.empty((B, T, C), dtype=np.float32)
    for b in range(B):
        y[b] = res.results[2 * b]["out"] + res.results[2 * b + 1]["out"] + bp
    return y
